# revision 1
# baseline (speedup 1.0000x reference)
"""Trainium2 Bass kernel for nn_ApplicationScoringLayer.

Computes out[l, r] = log_softmax(ts+bias, axis=vocab)[:, lhs_idx[l]] log-matmul-exp
log_softmax(rts[rhs_idx], axis=types) -> [L, R], via the algebraic restructuring:

    sa[r, k]  = softmax(rts[rhs_idx[r], :])[k]            (A side, row softmax over K=64)
    eb[k, l]  = exp(ts[k, lhs_idx[l]] + bias[lhs_idx[l]] - lnZ_k)
    Z_k       = sum_i exp(ts[k, i] + bias[i])             (full-vocab row sum)
    out[l, r] = ln( sum_k eb[k, l] * sa[r, k] )

All values stay comfortably inside fp32 range (|ts|<0.3, |bias|<6), so the
reference's max-subtraction is unnecessary; the result matches to fp32 rounding.

Sharding (8 cores):
  - output rows (lhs idxs) data-parallel: core c computes out[c*1024:(c+1)*1024, :]
  - Z pass vocab-sharded (each core sums exp over vocab/8)
  - A side r-sharded: each core gathers+softmaxes+transposes 1024 rts rows
  - ONE AllReduce(add) merges both: each core scatters its [64, 1025] block
    (sa^T part + its Z-partial as column 1024) into a zero-initialized k-major
    [512, 1025] bounce at rows k*8+c; the sum then holds every block and all
    Z partials in a layout that reloads as a single contiguous [64, 8200] DMA.
  - gather tables (rts row-major, tsTb = ts^T with bias column) replicated

HW notes baked in here:
  - indirect DMA moves one contiguous run per offset-AP partition (<=128
    runs per call); per-element indexed gathers are not a thing on HW.
  - PE fp32 matmul rounds one operand to ~bf16, so both operands are split
    hi/lo in bf16 and 3 exact bf16 products accumulate in PSUM (rel ~2^-16).
  - partition bias broadcast is done by stride-0-source DMA from DRAM.
  - HWDGE sequencer cost is per-descriptor (~45ns) -> big contiguous
    per-partition runs, few DMAs.
"""
import os
import sys

for _p in ("/opt/trn_rl_repo", os.path.expanduser("~/.axon_site/_ro/trn_rl_repo")):
    if os.path.isdir(_p) and _p not in sys.path:
        sys.path.insert(0, _p)

import contextlib

import numpy as np

import concourse.bacc as bacc
import concourse.bass as bass
import concourse.tile as tile
from concourse import mybir
from concourse.bass import IndirectOffsetOnAxis
from concourse.bass_utils import run_bass_kernel_spmd
from concourse.masks import make_identity
from concourse.tile import add_dep_helper

F32 = mybir.dt.float32
BF16 = mybir.dt.bfloat16
I32 = mybir.dt.int32
AF = mybir.ActivationFunctionType
ALU = mybir.AluOpType

V = 100000   # vocab size (both tables)
K = 64       # num types
R = 8192     # num rhs idxs
L = 8192     # num lhs idxs
N_CORES = 8
LS = L // N_CORES  # lhs idxs per core


def _pick_ztile(vs):
    for t in range(2560, 0, -1):
        if vs % t == 0:
            return t
    return vs


def build(v=V, k=K, r=R, l=L, n_cores=N_CORES, repeat=1, loop=1):
    """Build the SPMD Bass program (same NEFF on all cores)."""
    ls = l // n_cores            # output rows per core
    rs = r // n_cores            # A-side rows per core
    vs = v // n_cores            # Z-pass vocab per core
    bw = rs + 1                  # block width in the merged bounce (sa + z col)
    assert k <= 64 and rs % 512 == 0 and ls % 128 == 0
    nc = bacc.Bacc("TRN2", target_bir_lowering=False, debug=False,
                   num_devices=n_cores)

    rts = nc.dram_tensor("rts", [v, k], F32, kind="ExternalInput")
    # ts transposed with the bias appended as column k (host-marshalled layout)
    tsTb = nc.dram_tensor("tsTb", [v, k + 1], F32, kind="ExternalInput")
    ts_sh = nc.dram_tensor("ts_sh", [k, vs], F32, kind="ExternalInput")
    bias_sh = nc.dram_tensor("bias_sh", [1, vs], F32, kind="ExternalInput")
    # idx[:, 0:ga] = a_idx, idx[:, ga:ga+gb] = b_idx; [p, g] = idx[g*128+p]
    ga, gb = rs // 128, ls // 128
    gidx = nc.dram_tensor("gidx", [128, ga + gb], I32, kind="ExternalInput")
    out = nc.dram_tensor("out", [ls, r], F32, kind="ExternalOutput")

    groups = [list(range(n_cores))]
    hs = vs // 2                 # Z halves stacked on partitions 0-63 / 64-127
    zt = _pick_ztile(hs)
    nzt = hs // zt

    with tile.TileContext(nc) as tc:
        with (
            tc.tile_pool(name="persist", bufs=1) as pp,
            tc.tile_pool(name="zstream", bufs=3) as zp,
            tc.tile_pool(name="abig", bufs=1) as ap_,
            tc.tile_pool(name="small", bufs=1) as sp,
            tc.tile_pool(name="ostage", bufs=3) as op_,
            tc.tile_pool(name="ps", bufs=2, space="PSUM") as ps,
            tc.tile_pool(name="dram", bufs=1, space="DRAM") as dp,
        ):
            ident = pp.tile([128, 128], F32)
            make_identity(nc, ident[:])
            idx_sb = pp.tile([128, ga + gb], I32, tag="gidx")
            nc.sync.dma_start(idx_sb[:], gidx[:])

            loop_ctx = tc.For_i(0, loop, 1) if loop > 1 else contextlib.nullcontext()
            with loop_ctx:
              for _rep in range(repeat):
                # ------- Z pass (vocab shard): two halves stacked -------------
                zpart = pp.tile([128, nzt], F32, tag="zpart")
                z_exps = []
                for i in range(nzt):
                    tst = zp.tile([128, zt], F32, tag="tst")
                    nc.sync.dma_start(
                        tst[:], bass.AP(ts_sh, i * zt, [[hs, 2], [vs, k], [1, zt]]))
                    brep = zp.tile([128, zt], F32, tag="brep")
                    nc.scalar.dma_start(
                        brep[0:64, :],
                        bass.AP(bias_sh, i * zt, [[0, 64], [1, zt]]))
                    nc.scalar.dma_start(
                        brep[64:128, :],
                        bass.AP(bias_sh, hs + i * zt, [[0, 64], [1, zt]]))
                    tsb = zp.tile([128, zt], F32, tag="tsb")
                    nc.vector.tensor_tensor(out=tsb[:], in0=tst[:], in1=brep[:],
                                            op=ALU.add)
                    z_exps.append(nc.scalar.activation(
                        tsb[:], tsb[:], AF.Exp, accum_out=zpart[:, i:i + 1]))

                zsum = pp.tile([128, 1], F32, tag="zsum")
                zred = nc.vector.reduce_sum(zsum[:], zpart[:],
                                            axis=mybir.AxisListType.X)
                for e in z_exps:
                    add_dep_helper(zred.ins, e.ins, sync=True,
                                   reason="zsum waits on all zpart accum cols")
                zhi = pp.tile([64, 1], F32, tag="zhi")
                nc.sync.dma_start(zhi[:], zsum[64:128, :])
                zpar64 = pp.tile([64, 1], F32, tag="zpar64")
                nc.vector.tensor_tensor(out=zpar64[:], in0=zsum[0:64, :],
                                        in1=zhi[:], op=ALU.add)

                # ------- A side (r shard): gather 128 rows/call, softmax, T ---
                ea = ap_.tile([128, ga * k], F32, tag="ea")
                for g in range(ga):
                    nc.gpsimd.indirect_dma_start(
                        out=ea[:, g * k:(g + 1) * k], out_offset=None, in_=rts[:],
                        in_offset=IndirectOffsetOnAxis(
                            ap=idx_sb[:, g:g + 1], axis=0))
                nc.scalar.activation(ea[:], ea[:], AF.Exp)
                ea3 = ea[:].rearrange("p (g c) -> p g c", c=k)
                rsum = sp.tile([128, ga], F32, tag="rsum")
                nc.vector.reduce_sum(rsum[:], ea3, axis=mybir.AxisListType.X)
                rrec = sp.tile([128, ga], F32, tag="rrec")
                nc.vector.reciprocal(rrec[:], rsum[:])
                nc.vector.tensor_tensor(out=ea3, in0=ea3,
                                        in1=rrec[:].to_broadcast([128, ga, k]),
                                        op=ALU.mult)

                sapart = sp.tile([64, bw], F32, tag="sapart")
                nc.vector.tensor_copy(sapart[:, 0:1], zpar64[:])
                for g4 in range(0, ga, 4):
                    gn = min(4, ga - g4)
                    pst = ps.tile([64, 512], F32, tag="m")
                    for j in range(gn):
                        nc.tensor.transpose(
                            out=pst[0:k, j * 128:(j + 1) * 128],
                            in_=ea[:, (g4 + j) * k:(g4 + j + 1) * k],
                            identity=ident[:])
                    nc.vector.tensor_copy(sapart[:, 1 + g4 * 128:1 + (g4 + gn) * 128],
                                          pst[0:k, 0:gn * 128])

                # AllGather in 2 chunks (z + first sa half unblocks early),
                # rank-major blocks de-strided on load into per-chunk tiles
                w0 = 1 + min(rs, 512)          # chunk 0: z col + first 512 sa
                w1 = bw - w0                   # chunk 1: rest of the sa cols
                sa_in0 = dp.tile([64, w0], F32, tag="sa_in0")
                sa_out0 = dp.tile([n_cores, 64, w0], F32, tag="sa_out0")
                nc.sync.dma_start(sa_in0[:], sapart[:, 0:w0])
                cc0 = nc.gpsimd.collective_compute(
                    "AllGather", ALU.bypass, replica_groups=groups,
                    ins=[sa_in0[:]], outs=[sa_out0[:]])
                saT0 = ap_.tile([64, n_cores * w0], F32, tag="saT0")
                nc.sync.dma_start(
                    saT0[:], bass.AP(sa_out0.tensor, sa_out0[:].offset,
                                     [[w0, 64], [64 * w0, n_cores], [1, w0]]))
                if w1:
                    sa_in1 = dp.tile([64, w1], F32, tag="sa_in1")
                    sa_out1 = dp.tile([n_cores, 64, w1], F32, tag="sa_out1")
                    nc.sync.dma_start(sa_in1[:], sapart[:, w0:bw])
                    cc1 = nc.gpsimd.collective_compute(
                        "AllGather", ALU.bypass, replica_groups=groups,
                        ins=[sa_in1[:]], outs=[sa_out1[:]])
                    add_dep_helper(cc1.ins, cc0.ins, sync=True,
                                   reason="chunk-0 gather (carries Z) first")
                    saT1 = ap_.tile([64, n_cores * w1], F32, tag="saT1")
                    nc.sync.dma_start(
                        saT1[:], bass.AP(sa_out1.tensor, sa_out1[:].offset,
                                         [[w1, 64], [64 * w1, n_cores], [1, w1]]))
                # Z = sum of the gathered partials (block-leading columns)
                z64 = pp.tile([64, 1], F32, tag="z64")
                nc.vector.reduce_sum(
                    z64[:], bass.AP(saT0.tensor, saT0[:].offset,
                                    [[saT0[:].ap[0][0], 64], [w0, n_cores]]),
                    axis=mybir.AxisListType.X)
                nlz = pp.tile([64, 1], F32, tag="nlz")
                nc.scalar.activation(nlz[:], z64[:], AF.Ln)
                nc.vector.tensor_scalar_mul(nlz[:], nlz[:], -1.0)

                # ------- B side: eb [k, ls] from tsTb row gathers -------------
                tsbg = sp.tile([128, gb * (k + 1)], F32, tag="tsbg")
                for g in range(gb):
                    nc.gpsimd.indirect_dma_start(
                        out=tsbg[:, g * (k + 1):(g + 1) * (k + 1)],
                        out_offset=None, in_=tsTb[:],
                        in_offset=IndirectOffsetOnAxis(
                            ap=idx_sb[:, ga + g:ga + g + 1], axis=0))
                tsg3 = tsbg[:].rearrange("p (g c) -> p g c", c=k + 1)
                tsb2 = sp.tile([128, gb * k], F32, tag="tsb2")
                badd = nc.vector.tensor_tensor(
                    out=tsb2[:].rearrange("p (g c) -> p g c", c=k),
                    in0=tsg3[:, :, 0:k],
                    in1=tsg3[:, :, k:k + 1].to_broadcast([128, gb, k]),
                    op=ALU.add)
                # keep the B-side off the DVE queue head until Z is reduced
                add_dep_helper(badd.ins, zred.ins, sync=True,
                               reason="B-side add after Z reduce (DVE HOL)")
                ebp = sp.tile([k, ls], F32, tag="ebp")
                for g2 in range(0, gb, 4):
                    gn = min(4, gb - g2)
                    pst = ps.tile([64, 512], F32, tag="m")
                    for j in range(gn):
                        nc.tensor.transpose(
                            out=pst[0:k, j * 128:(j + 1) * 128],
                            in_=tsb2[:, (g2 + j) * k:(g2 + j + 1) * k],
                            identity=ident[:])
                    nc.vector.tensor_copy(ebp[:, g2 * 128:(g2 + gn) * 128],
                                          pst[0:k, 0:gn * 128])
                nc.scalar.activation(ebp[:], ebp[:], AF.Exp, bias=nlz[:])

                # ------- matmul + ln + store ----------------------------------
                # j-phased: all chunk-0 slices (rhs in saT0) first across every
                # m so the PE never queues behind chunk-1's collective.
                jn = rs // 512                  # 512-slices per block
                jsplit = max(1, min(rs, 512) // 512)  # slices served by saT0
                phases = [(0, jsplit)] + ([(jsplit, jn)] if jn > jsplit else [])
                for j0, j1 in phases:
                    nslice = (j1 - j0) * n_cores         # 512-slices this phase
                    for m in range(ls // 128):
                        msl = slice(m * 128, (m + 1) * 128)
                        ot = op_.tile([128, nslice * 512], F32, tag="ot")
                        for cg in range(0, nslice, 4):   # 4 slices per PSUM
                            cn = min(4, nslice - cg)
                            pst = ps.tile([128, 2048], F32, tag="m")
                            for s in range(cn):
                                c, j = divmod(cg + s, j1 - j0)
                                j += j0
                                if j < jsplit:
                                    rhs = saT0[:, c * w0 + 1 + j * 512:
                                               c * w0 + 1 + j * 512 + 512]
                                else:
                                    rhs = saT1[:, c * w1 + (j - jsplit) * 512:
                                               c * w1 + (j - jsplit) * 512 + 512]
                                nc.tensor.matmul(pst[:, s * 512:(s + 1) * 512],
                                                 lhsT=ebp[:, msl], rhs=rhs,
                                                 start=True, stop=True)
                            nc.scalar.activation(
                                ot[:, cg * 512:(cg + cn) * 512],
                                pst[:, 0:cn * 512], AF.Ln)
                        # strided store: this phase's j-slices of each block
                        nc.sync.dma_start(
                            bass.AP(out, m * 128 * r + j0 * 512,
                                    [[r, 128], [rs, n_cores],
                                     [1, (j1 - j0) * 512]]),
                            ot[:])
    nc.compile()
    return nc


def make_in_maps(rhs_type_scores, type_lhs_scores, lhs_nonterminal_bias,
                 rhs_emb_idxs, lhs_emb_idxs, v=V, k=K, r=R, n_cores=N_CORES):
    """Host-side input marshalling: replicate gather tables (tsTb = ts^T with
    the bias as an extra column), shard lhs/rhs idx lists and the Z-pass
    vocab range across cores."""
    l = len(lhs_emb_idxs)
    ls, rs, vs = l // n_cores, r // n_cores, v // n_cores
    rts_np = np.ascontiguousarray(np.asarray(rhs_type_scores, dtype=np.float32))
    ts_np = np.ascontiguousarray(np.asarray(type_lhs_scores, dtype=np.float32))
    bias_np = np.ascontiguousarray(
        np.asarray(lhs_nonterminal_bias, dtype=np.float32).reshape(1, v))
    tsTb_np = np.ascontiguousarray(
        np.concatenate([ts_np.T, bias_np.T], axis=1))  # [v, k+1]
    ridx = np.asarray(rhs_emb_idxs, dtype=np.int64)
    lidx = np.asarray(lhs_emb_idxs, dtype=np.int64)
    in_maps = []
    for c in range(n_cores):
        lsh = lidx[c * ls:(c + 1) * ls]
        rsh = ridx[c * rs:(c + 1) * rs]
        gidx = np.concatenate([
            rsh.reshape(rs // 128, 128).T,   # [p, g] = idx[g*128 + p]
            lsh.reshape(ls // 128, 128).T,
        ], axis=1).astype(np.int32)
        in_maps.append({
            "rts": rts_np, "tsTb": tsTb_np,
            "ts_sh": np.ascontiguousarray(ts_np[:, c * vs:(c + 1) * vs]),
            "bias_sh": np.ascontiguousarray(bias_np[:, c * vs:(c + 1) * vs]),
            "gidx": np.ascontiguousarray(gidx),
        })
    return in_maps


def kernel(rhs_type_scores, type_lhs_scores, lhs_nonterminal_bias,
           rhs_emb_idxs, lhs_emb_idxs):
    nc = build()
    in_maps = make_in_maps(rhs_type_scores, type_lhs_scores,
                           lhs_nonterminal_bias, rhs_emb_idxs, lhs_emb_idxs)
    res = run_bass_kernel_spmd(nc, in_maps, core_ids=list(range(N_CORES)))
    return np.concatenate([res.results[c]["out"] for c in range(N_CORES)],
                          axis=0)



# revision 2
# speedup vs baseline: 23.2007x; 23.2007x over previous
"""Trainium2 Bass kernel for nn_ApplicationScoringLayer (optimized v4).

out[l, r] = ln( sum_k eb[k, l] * sa[r, k] ),
  sa[r, :] = softmax(rts[rhs_idx[r], :])                  (row softmax over K=64)
  eb[k, l] = exp(tsb[k, lhs_idx[l]]) / Z_k,   tsb = ts + bias (host pre-added)
  Z_k      = sum_v exp(tsb[k, v])                         (full-vocab row sum)

Sharding (8 cores): output rows (lhs idxs) data-parallel (1024 rows/core);
Z pass vocab-sharded; A side r-sharded then ONE bf16 AllGather ships every
core's sa^T block + Z partial column.

Pipelining: the loop is ROTATED — iteration i+1's whole prologue (Z pass,
A side, B side, collective launch + reload) is emitted BEFORE iteration i's
main loop, so every engine's in-order queue sees next-iteration prologue work
before the current matmul/Ln/store stream:
  ACT: [exps(i+1), Lns(i)]          (no collective-dependent op on ACT at all:
                                     eb uses DVE reciprocal(Z) * exp, not
                                     exp(.-lnZ))
  PE:  [transposes(i+1), mms(i)]
  Pool:[gathers(i+1), cc(i+1), reload(i+1)]  (collective + its DMAs off SP)
  SP:  [z-stream(i+1), stores(i)]
  DVE: [softmax/copies(i+1), z-recip(i), eb-scale(i), ...]

Other speed choices: bf16 matmul operands (fp32 PE matmul is 4x slower),
fp16 output staging + store (halves the dominant HBM write, and 16-bit ACT
writes are ~1.35x faster than fp32), bf16 Z-stream, bias pre-added on host,
single-DMA-per-m contiguous stores.
"""
import os
import sys

for _p in ("/opt/trn_rl_repo", os.path.expanduser("~/.axon_site/_ro/trn_rl_repo")):
    if os.path.isdir(_p) and _p not in sys.path:
        sys.path.insert(0, _p)

import ml_dtypes
import numpy as np

import concourse.bacc as bacc
import concourse.bass as bass
import concourse.tile as tile
from concourse import mybir
from concourse.bass import IndirectOffsetOnAxis
from concourse.bass_utils import run_bass_kernel_spmd
from concourse.masks import make_identity
from concourse.tile import add_dep_helper

F32 = mybir.dt.float32
F16 = mybir.dt.float16
BF16 = mybir.dt.bfloat16
I32 = mybir.dt.int32
AF = mybir.ActivationFunctionType
ALU = mybir.AluOpType

V = 100000   # vocab size (both tables)
K = 64       # num types
R = 8192     # num rhs idxs
L = 8192     # num lhs idxs
N_CORES = 8
LS = L // N_CORES


def _pick_ztile(vs):
    for t in range(2560, 0, -1):
        if vs % t == 0:
            return t
    return vs


def build(v=V, k=K, r=R, l=L, n_cores=N_CORES, repeat=1):
    """Build the SPMD Bass program (same NEFF on all cores)."""
    ls = l // n_cores            # output rows per core
    rs = r // n_cores            # A-side rows per core
    vs = v // n_cores            # Z-pass vocab per core
    bw = rs + 1                  # sa^T block width + z column
    assert k <= 64 and rs % 512 == 0 and ls % 128 == 0
    nc = bacc.Bacc("TRN2", target_bir_lowering=False, debug=False,
                   num_devices=n_cores)

    rts = nc.dram_tensor("rts", [v, k], F32, kind="ExternalInput")
    tsTbB = nc.dram_tensor("tsTbB", [v, k], F32, kind="ExternalInput")
    tsb_sh = nc.dram_tensor("tsb_sh", [k, vs], BF16, kind="ExternalInput")
    ga, gb = rs // 128, ls // 128
    gidx = nc.dram_tensor("gidx", [128, ga + gb], I32, kind="ExternalInput")
    out = nc.dram_tensor("out", [ls, r], F16, kind="ExternalOutput")

    groups = [list(range(n_cores))]
    hs = vs // 2                 # Z halves stacked on partitions 0-63 / 64-127
    zt = _pick_ztile(hs)
    nzt = hs // zt
    nj = rs // 512               # 512-col matmul slices per sa block

    with tile.TileContext(nc) as tc:
        with (
            tc.tile_pool(name="persist", bufs=1) as pp,
            tc.tile_pool(name="pipe", bufs=2) as qp,
            tc.tile_pool(name="zstream", bufs=3) as zp,
            tc.tile_pool(name="abig", bufs=2) as ap_,
            tc.tile_pool(name="ostage", bufs=3) as op_,
            tc.tile_pool(name="ps", bufs=2, space="PSUM") as ps,
            tc.tile_pool(name="dram", bufs=2, space="DRAM") as dp,
        ):
            ident = pp.tile([128, 128], F32)
            make_identity(nc, ident[:])
            idx_sb = pp.tile([128, ga + gb], I32, tag="gidx")
            nc.sync.dma_start(idx_sb[:], gidx[:])

            def prologue():
                """Emit Z pass + A side + B side + collective for one
                iteration; returns the state the main loop consumes."""
                # ---- Z pass (vocab shard, two halves stacked) ----
                zpart = qp.tile([128, nzt], F32, tag="zpart", name="zpart")
                z_exps = []
                for i in range(nzt):
                    tst = zp.tile([128, zt], BF16, tag="tst", name="tst")
                    nc.sync.dma_start(
                        tst[:], bass.AP(tsb_sh, i * zt, [[hs, 2], [vs, k], [1, zt]]))
                    z_exps.append(nc.scalar.activation(
                        tst[:], tst[:], AF.Exp, accum_out=zpart[:, i:i + 1]))
                zsum = qp.tile([128, 1], F32, tag="zsum", name="zsum")
                zred = nc.vector.reduce_sum(zsum[:], zpart[:],
                                            axis=mybir.AxisListType.X)
                for e in z_exps:
                    add_dep_helper(zred.ins, e.ins, sync=True,
                                   reason="zsum waits on all zpart accum cols")
                zhi = qp.tile([64, 1], F32, tag="zhi", name="zhi")
                nc.sync.dma_start(zhi[:], zsum[64:128, :])
                zpar64 = qp.tile([64, 1], F32, tag="zpar64", name="zpar64")
                nc.vector.tensor_tensor(out=zpar64[:], in0=zsum[0:64, :],
                                        in1=zhi[:], op=ALU.add)

                # ---- A side: gather, softmax, transpose into sapart ----
                ea = ap_.tile([128, ga * k], F32, tag="ea", name="ea")
                for g in range(ga):
                    nc.gpsimd.indirect_dma_start(
                        out=ea[:, g * k:(g + 1) * k], out_offset=None, in_=rts[:],
                        in_offset=IndirectOffsetOnAxis(
                            ap=idx_sb[:, g:g + 1], axis=0))
                nc.scalar.activation(ea[:], ea[:], AF.Exp)
                ea3 = ea[:].rearrange("p (g c) -> p g c", c=k)
                rsum = qp.tile([128, ga], F32, tag="rsum", name="rsum")
                nc.vector.reduce_sum(rsum[:], ea3, axis=mybir.AxisListType.X)
                rrec = qp.tile([128, ga], F32, tag="rrec", name="rrec")
                nc.vector.reciprocal(rrec[:], rsum[:])
                nc.vector.tensor_tensor(out=ea3, in0=ea3,
                                        in1=rrec[:].to_broadcast([128, ga, k]),
                                        op=ALU.mult)

                sapart = qp.tile([64, bw], BF16, tag="sapart", name="sapart")
                nc.vector.tensor_copy(sapart[:, 0:1], zpar64[:])
                for g4 in range(0, ga, 4):
                    gn = min(4, ga - g4)
                    pst = ps.tile([64, 512], F32, tag="m", name="pst")
                    for j in range(gn):
                        nc.tensor.transpose(
                            out=pst[0:k, j * 128:(j + 1) * 128],
                            in_=ea[:, (g4 + j) * k:(g4 + j + 1) * k],
                            identity=ident[:])
                    nc.vector.tensor_copy(sapart[:, 1 + g4 * 128:1 + (g4 + gn) * 128],
                                          pst[0:k, 0:gn * 128])

                # ---- B side (emitted before the collective so nothing here
                #      queues behind it): gather, transpose, exp ----
                tsbg = qp.tile([128, gb * k], F32, tag="tsbg", name="tsbg")
                for g in range(gb):
                    nc.gpsimd.indirect_dma_start(
                        out=tsbg[:, g * k:(g + 1) * k],
                        out_offset=None, in_=tsTbB[:],
                        in_offset=IndirectOffsetOnAxis(
                            ap=idx_sb[:, ga + g:ga + g + 1], axis=0))
                ebf = qp.tile([k, ls], F32, tag="ebf", name="ebf")
                for g2 in range(0, gb, 4):
                    gn = min(4, gb - g2)
                    pst = ps.tile([64, 512], F32, tag="m", name="pst")
                    for j in range(gn):
                        nc.tensor.transpose(
                            out=pst[0:k, j * 128:(j + 1) * 128],
                            in_=tsbg[:, (g2 + j) * k:(g2 + j + 1) * k],
                            identity=ident[:])
                    nc.vector.tensor_copy(ebf[:, g2 * 128:(g2 + gn) * 128],
                                          pst[0:k, 0:gn * 128])
                et = qp.tile([k, ls], F32, tag="et", name="et")
                nc.scalar.activation(et[:], ebf[:], AF.Exp)

                # ---- AllGather (single, bf16, Shared output) on Pool ----
                ci = dp.tile([64, bw], BF16, tag="ci", name="ci")
                co = dp.tile([n_cores, 64, bw], BF16, tag="co", name="co",
                             addr_space="Shared")
                nc.gpsimd.dma_start(ci[:], sapart[:])
                nc.gpsimd.collective_compute(
                    "AllGather", ALU.bypass, replica_groups=groups,
                    ins=[ci[:]], outs=[co[:]])
                saT = ap_.tile([64, n_cores * bw], BF16, tag="saT", name="saT")
                nc.gpsimd.dma_start(
                    saT[:], bass.AP(co.tensor, co[:].offset,
                                    [[bw, 64], [64 * bw, n_cores], [1, bw]]))
                return saT, et

            def mainloop(saT, et):
                # Z total from the gathered per-core partial columns, then
                # eb = exp(tsb_gathered) / Z — all off the ACT queue.
                z64 = qp.tile([64, 1], F32, tag="z64", name="z64")
                nc.vector.reduce_sum(
                    z64[:], bass.AP(saT.tensor, saT[:].offset,
                                    [[saT[:].ap[0][0], 64], [bw, n_cores]]),
                    axis=mybir.AxisListType.X)
                rz = qp.tile([64, 1], F32, tag="rz", name="rz")
                nc.vector.reciprocal(rz[:], z64[:])
                ebp = qp.tile([k, ls], BF16, tag="ebp", name="ebp")
                nc.vector.tensor_tensor(out=ebp[:], in0=et[:],
                                        in1=rz[:].to_broadcast([k, ls]),
                                        op=ALU.mult)

                # saT col t = c*bw + 1 + j*512 + jj  <->  out col c*rs + j*512 + jj
                for m in range(ls // 128):
                    msl = slice(m * 128, (m + 1) * 128)
                    ot = op_.tile([128, r], F16, tag="ot", name="ot")
                    for cg in range(0, n_cores * nj, 4):
                        pst = ps.tile([128, 2048], F32, tag="m", name="pst")
                        for s in range(4):
                            c, j = divmod(cg + s, nj)
                            nc.tensor.matmul(
                                pst[:, s * 512:(s + 1) * 512],
                                lhsT=ebp[:, msl],
                                rhs=saT[:, c * bw + 1 + j * 512:
                                        c * bw + 1 + j * 512 + 512],
                                start=True, stop=True)
                        nc.scalar.activation(
                            ot[:, cg * 512:(cg + 4) * 512],
                            pst[:], AF.Ln)
                    nc.sync.dma_start(
                        bass.AP(out, m * 128 * r, [[r, 128], [1, r]]),
                        ot[:])

            state = prologue()
            for i in range(repeat):
                nxt = prologue() if i + 1 < repeat else None
                mainloop(*state)
                state = nxt
    nc.compile()
    return nc


def make_in_maps(rhs_type_scores, type_lhs_scores, lhs_nonterminal_bias,
                 rhs_emb_idxs, lhs_emb_idxs, v=V, k=K, r=R, n_cores=N_CORES):
    """Host-side input marshalling: bias pre-added into both B-side layouts,
    gather tables replicated, idx lists and the Z-pass vocab range sharded."""
    l = len(lhs_emb_idxs)
    ls, rs, vs = l // n_cores, r // n_cores, v // n_cores
    rts_np = np.ascontiguousarray(np.asarray(rhs_type_scores, dtype=np.float32))
    ts_np = np.asarray(type_lhs_scores, dtype=np.float32)
    bias_np = np.asarray(lhs_nonterminal_bias, dtype=np.float32).reshape(1, v)
    tsb_np = ts_np + bias_np                                   # [k, v]
    tsTbB_np = np.ascontiguousarray(tsb_np.T)                  # [v, k]
    ridx = np.asarray(rhs_emb_idxs, dtype=np.int64)
    lidx = np.asarray(lhs_emb_idxs, dtype=np.int64)
    in_maps = []
    for c in range(n_cores):
        lsh = lidx[c * ls:(c + 1) * ls]
        rsh = ridx[c * rs:(c + 1) * rs]
        gidx = np.concatenate([
            rsh.reshape(rs // 128, 128).T,   # [p, g] = idx[g*128 + p]
            lsh.reshape(ls // 128, 128).T,
        ], axis=1).astype(np.int32)
        in_maps.append({
            "rts": rts_np, "tsTbB": tsTbB_np,
            "tsb_sh": np.ascontiguousarray(
                tsb_np[:, c * vs:(c + 1) * vs]).astype(ml_dtypes.bfloat16),
            "gidx": np.ascontiguousarray(gidx),
        })
    return in_maps


def kernel(rhs_type_scores, type_lhs_scores, lhs_nonterminal_bias,
           rhs_emb_idxs, lhs_emb_idxs):
    nc = build()
    in_maps = make_in_maps(rhs_type_scores, type_lhs_scores,
                           lhs_nonterminal_bias, rhs_emb_idxs, lhs_emb_idxs)
    res = run_bass_kernel_spmd(nc, in_maps, core_ids=list(range(N_CORES)))
    return np.concatenate(
        [np.asarray(res.results[c]["out"]).astype(np.float32)
         for c in range(N_CORES)], axis=0)
